# revision 3
# baseline (speedup 1.0000x reference)
"""MoE (DbrxExperts) expert-parallel Trainium2 kernel.

Strategy:
  - Host: compute per-(expert,token) combine weights cw from top_experts /
    top_weights, gather each expert's routed tokens, and pre-transpose
    operands so the device kernel needs no on-chip transposes.
  - Experts are sorted by routed-token count and rank-paired onto cores:
    core m gets slot0 = rank[m] (big), slot1 = rank[15-m] (small). Slot
    capacities C0 = max big count, C1 = max small count are EXACT (no
    128 padding) and baked into the compiled program, so per-core PE work
    is C0+C1 token-columns instead of 2*pad128(max count).
  - Device (8 cores, SPMD): per slot
        gate_T = W1T_blocks^T @ XT     [F, C]   (contract H)
        up_T   = V1T_blocks^T @ XT     [F, C]
        hact_T = silu(gate_T) * up_T   [F, C]   (ACT + DVE)
        down   = hact_T_blocks^T @ W2  [C, H]   (contract F)
    All fp32 data, fp32r matmuls. Output in natural [C, H] layout.
  - Host: out[tokens_e] += down_e * cw_e  (scaling folded into combine).
"""

import numpy as np
from contextlib import ExitStack

N_CORES = 8
B, S, H = 4, 2048, 1024
F, E = 2048, 16
T = B * S
N_SLOTS = 2  # experts per core

P = 128
HT = H // P   # 8  h-tiles
FT = F // P   # 16 f-tiles

TRACE = False          # test.py sets this for profiled runs
TRACE_CORES = [7]      # core-0 NTFF capture crashes fast kernels here
MM_DTYPE = "fp32r"     # "fp32" | "fp32r" (tf32-rate, 4x PE) | "bf16"
LAST_RESULT = None     # BassKernelResults of last run (for test.py)

_nc_cache = {}


def _parts(s):
    """Split s columns into matmul parts, each <=512 and >=256 where
    possible (fp32r needs moving dim >=256 for full rate)."""
    out = []
    while s > 768:
        out.append(512)
        s -= 512
    if s > 512:
        out.append(s - 256)
        out.append(256)
    elif s > 0:
        out.append(s)
    return out


def _plan(C, warm):
    """Chunk sizes for a slot of capacity C. Chunks <=1024 (SBUF budget);
    all but the last are multiples of 128 so GEMM3's 128-token blocks
    stay aligned and the ceil(C/128) block count is not inflated.
    warm=True prepends small 256/512 chunks so the very first matmuls
    only wait on a small DMA fill."""
    sizes = []
    rem = C
    if warm and C > 1792:
        sizes += [256, 512]
        rem -= 768
    while rem > 0:
        n = -(-rem // 1024)
        if n == 1:
            sizes.append(rem)
            rem = 0
        else:
            piece = min(1024, -(-(rem // n) // 128) * 128)
            # keep the eventual final piece >= 256
            if 0 < rem - piece < 256:
                piece -= 128
            sizes.append(piece)
            rem -= piece
    out = []
    c0 = 0
    for s_ in sizes:
        out.append((c0, s_, _parts(s_)))
        c0 += s_
    return out


def _build_nc(caps):
    # NOTE: reads module-global MM_DTYPE
    import concourse.tile as tile
    from concourse import bacc, mybir

    nc = bacc.Bacc("TRN2", target_bir_lowering=False, debug=False,
                   enable_asserts=False, num_devices=N_CORES)
    dt = mybir.dt.float32
    mdt = {"fp32": mybir.dt.float32, "fp32r": mybir.dt.float32r,
           "bf16": mybir.dt.bfloat16}[MM_DTYPE]
    mm = lambda ap: ap
    SILU = mybir.ActivationFunctionType.Silu

    xts, w1ts, v1ts, w2s, ys = [], [], [], [], []
    for s in range(N_SLOTS):
        C = caps[s]
        xts.append(nc.dram_tensor(f"xt{s}", [H, C], mdt,
                                  kind="ExternalInput").ap())
        # w1t/v1t arrive pre-blocked: [ft, p(h%128), o(h//128), f] so each
        # ft slice is contiguous and DMAs as 128 x 4KB descriptors
        w1ts.append(nc.dram_tensor(f"w1t{s}", [FT, P, HT, P], mdt,
                                   kind="ExternalInput").ap())
        v1ts.append(nc.dram_tensor(f"v1t{s}", [FT, P, HT, P], mdt,
                                   kind="ExternalInput").ap())
        w2s.append(nc.dram_tensor(f"w2_{s}", [F, H], mdt,
                                  kind="ExternalInput").ap())
        ys.append(nc.dram_tensor(f"y{s}", [C, H], dt,
                                 kind="ExternalOutput").ap())

    with tile.TileContext(nc) as tc:
        with ExitStack() as ctx:
            xt_pool = ctx.enter_context(tc.tile_pool(name="xt", bufs=HT))
            wst_pool = ctx.enter_context(tc.tile_pool(name="wst", bufs=4))
            w2_pool = ctx.enter_context(tc.tile_pool(name="w2sb", bufs=FT))
            hact_pool = ctx.enter_context(tc.tile_pool(name="hact", bufs=FT))
            silu_pool = ctx.enter_context(tc.tile_pool(name="silu", bufs=2))
            out_pool = ctx.enter_context(tc.tile_pool(name="out", bufs=2))
            ps_pool = ctx.enter_context(tc.tile_pool(name="ps", bufs=8, space="PSUM"))

            for s in range(N_SLOTS):
                C = caps[s]
                xt, w1t, v1t, w2, y = xts[s], w1ts[s], v1ts[s], w2s[s], ys[s]
                # W2 tiles for this slot are emitted after the first
                # chunk's GEMM1/2 so their DMAs don't delay the XT/W1T
                # loads the first matmuls depend on
                w2_sb = []

                for ci, (c0, S_, parts) in enumerate(_plan(C, warm=(s == 0))):
                    # On the very first chunk of the run, land ft0's weight
                    # tiles before the xt block so the first matmul only
                    # waits on ~1.5MB of DMA instead of the full chunk.
                    warm_wst = None
                    if s == 0 and ci == 0:
                        w1s0 = wst_pool.tile([P, HT, P], mdt, tag="wst")
                        v1s0 = wst_pool.tile([P, HT, P], mdt, tag="wst")
                        nc.sync.dma_start(w1s0[:], w1t[0])
                        nc.sync.dma_start(v1s0[:], v1t[0])
                        warm_wst = (w1s0, v1s0)

                    # XT chunk: 8 tiles [128, S_], partition = h within tile
                    xt_sb = []
                    for ht in range(HT):
                        t = xt_pool.tile([P, S_], mdt, tag="xt")
                        nc.sync.dma_start(
                            t[:], xt[ht * P:(ht + 1) * P, c0:c0 + S_])
                        xt_sb.append(t)

                    # GEMM1/2 + GLU -> hact_T tiles [128, S_] per f-tile
                    hact_sb = []
                    for ft in range(FT):
                        h_t = hact_pool.tile([P, S_], mdt, tag="hact")
                        if ft == 0 and warm_wst is not None:
                            w1s, v1s = warm_wst
                        else:
                            w1s = wst_pool.tile([P, HT, P], mdt, tag="wst")
                            v1s = wst_pool.tile([P, HT, P], mdt, tag="wst")
                            nc.sync.dma_start(w1s[:], w1t[ft])
                            nc.sync.dma_start(v1s[:], v1t[ft])
                        # all parts of this f-tile live at once so each
                        # LDWEIGHTS serves len(parts) matmuls per matrix
                        offs = []
                        o = 0
                        for p_ in parts:
                            offs.append((o, p_))
                            o += p_
                        g_tiles = [ps_pool.tile([P, p_], dt, tag="ps",
                                                name=f"g{i_}")
                                   for i_, (_, p_) in enumerate(offs)]
                        u_tiles = [ps_pool.tile([P, p_], dt, tag="ps",
                                               name=f"u{i_}")
                                   for i_, (_, p_) in enumerate(offs)]
                        for ht in range(HT):
                            for i_, (o_, p_) in enumerate(offs):
                                nc.tensor.matmul(
                                    g_tiles[i_][:], mm(w1s[:, ht, :]),
                                    mm(xt_sb[ht][:, o_:o_ + p_]),
                                    start=(ht == 0), stop=(ht == HT - 1))
                            for i_, (o_, p_) in enumerate(offs):
                                nc.tensor.matmul(
                                    u_tiles[i_][:], mm(v1s[:, ht, :]),
                                    mm(xt_sb[ht][:, o_:o_ + p_]),
                                    start=(ht == 0), stop=(ht == HT - 1))
                        for i_, (o_, p_) in enumerate(offs):
                            sl = silu_pool.tile([P, p_], dt, tag="sl")
                            nc.scalar.activation(sl[:], g_tiles[i_][:], SILU)
                            nc.vector.tensor_mul(
                                h_t[:, o_:o_ + p_], sl[:], u_tiles[i_][:])
                        hact_sb.append(h_t)

                    if ci == 0:
                        for ft in range(FT):
                            t = w2_pool.tile([P, H], mdt, tag="w2",
                                             name=f"w2_{ft}")
                            nc.sync.dma_start(
                                t[:], w2[ft * P:(ft + 1) * P, :])
                            w2_sb.append(t)

                    # GEMM3: down[c, h] accumulated over f-tiles; H split
                    # into two 512 halves so psum slots stay one bank each
                    for ct in range(-(-S_ // P)):
                        cp = min(P, S_ - ct * P)
                        o_t = out_pool.tile([P, H], dt, tag="o")
                        for hi, hh in enumerate(range(0, H, 512)):
                            d_ps = ps_pool.tile([P, 512], dt, tag="ps",
                                                name=f"d{hi}")
                            for ft in range(FT):
                                nc.tensor.matmul(
                                    d_ps[:cp, :],
                                    mm(hact_sb[ft][:, ct * P:ct * P + cp]),
                                    mm(w2_sb[ft][:, hh:hh + 512]),
                                    start=(ft == 0), stop=(ft == FT - 1))
                            nc.any.tensor_copy(o_t[:cp, hh:hh + 512],
                                               d_ps[:cp, :])
                            nc.sync.dma_start(
                                y[c0 + ct * P:c0 + ct * P + cp,
                                  hh:hh + 512],
                                o_t[:cp, hh:hh + 512])
    nc.compile()
    return nc


def _get_nc(caps):
    key = (caps, MM_DTYPE)
    if key not in _nc_cache:
        _nc_cache[key] = _build_nc(caps)
    return _nc_cache[key]


def prepare(x, top_weights, top_experts, w1, v1, w2):
    """Host-side routing + sharded input construction.
    Returns (caps, in_maps, slot_experts, idx, counts, cw)."""
    x = np.asarray(x, dtype=np.float32)
    top_weights = np.asarray(top_weights, dtype=np.float32)
    top_experts = np.asarray(top_experts).astype(np.int64)
    w1 = np.asarray(w1, dtype=np.float32)
    v1 = np.asarray(v1, dtype=np.float32)
    w2 = np.asarray(w2, dtype=np.float32)

    xf = x.reshape(T, H)

    # combine weights per (token, expert); duplicate slots sum
    cw = np.zeros((T, E), dtype=np.float32)
    np.add.at(cw, (np.arange(T)[:, None], top_experts), top_weights)

    idx = [np.nonzero(cw[:, e])[0] for e in range(E)]
    counts = [len(i) for i in idx]

    # sort experts by count desc; slot0 = ranks 0..7, slot1 = ranks 8..15;
    # core m gets (rank[m], rank[15-m]) so capacities are exact maxima
    order = sorted(range(E), key=lambda e: -counts[e])
    slot_experts = [[order[m] for m in range(N_CORES)],
                    [order[2 * N_CORES - 1 - m] for m in range(N_CORES)]]
    # fp32r ISA requires even moving/dst free sizes -> even capacities
    caps = tuple(max(256, (max(counts[e] for e in slot_experts[s]) + 1)
                 // 2 * 2) for s in range(N_SLOTS))

    def _block(we):
        # [F, H] -> [ft, p(h%128), o(h//128), f]: each ft slice
        # contiguous so the DMA runs 128 x 4KB descriptors
        wl = we.reshape(FT, P, HT, P)  # [ft, f, o, p]
        return np.ascontiguousarray(wl.transpose(0, 3, 2, 1))

    in_maps = []
    for m in range(N_CORES):
        im = {}
        for s in range(N_SLOTS):
            e = slot_experts[s][m]
            C = caps[s]
            XT = np.zeros((H, C), dtype=np.float32)
            XT[:, :counts[e]] = xf[idx[e]].T
            im[f"xt{s}"] = XT
            im[f"w1t{s}"] = _block(w1[e])
            im[f"v1t{s}"] = _block(v1[e])
            im[f"w2_{s}"] = np.ascontiguousarray(w2[e])
        if MM_DTYPE == "bf16":
            import ml_dtypes
            im = {k: v.astype(ml_dtypes.bfloat16) for k, v in im.items()}
        in_maps.append(im)
    return caps, in_maps, slot_experts, idx, counts, cw


def combine(results, slot_experts, idx, counts, cw):
    """Weighted scatter-add of per-core expert outputs into [B, S, H]."""
    out = np.zeros((T, H), dtype=np.float32)
    for m in range(N_CORES):
        for s in range(N_SLOTS):
            e = slot_experts[s][m]
            n = counts[e]
            if n:
                ym = results[m][f"y{s}"]
                out[idx[e]] += ym[:n, :] * cw[idx[e], e][:, None]
    return out.reshape(B, S, H)


def kernel(x, weights, top_weights, top_experts, w1, v1, w2):
    global LAST_RESULT
    caps, in_maps, slot_experts, idx, counts, cw = prepare(
        x, top_weights, top_experts, w1, v1, w2)
    nc = _get_nc(caps)
    from concourse.bass_utils import run_bass_kernel_spmd
    res = run_bass_kernel_spmd(nc, in_maps, list(range(N_CORES)), trace=TRACE,
                               trace_cores=TRACE_CORES if TRACE else None)
    LAST_RESULT = res
    return combine(res.results, slot_experts, idx, counts, cw)


# revision 4
# speedup vs baseline: 1.2378x; 1.2378x over previous
"""MoE (DbrxExperts) expert-parallel Trainium2 kernel, v3 (bf16, ft-outer).

Strategy:
  - Host: route tokens to experts, sort experts by count desc, rank-pair
    onto cores (core m: slot0 = rank[m], slot1 = rank[15-m]). Slot
    capacities C0/C1 are exact maxima, baked into the program.
  - All matmul operands bf16 (PE rate identical to fp32r; DMA halved;
    whole slot's xt + hact fit in SBUF). fp32 accumulation in PSUM.
  - Device, per slot (ft-outer so w1/v1 stream exactly once):
      xt (8 tiles [128, C]) DMA'd up front, consumed ht-progressively.
      for ft: load w1s/v1s [128, HT*128]; for each column-pair group:
        gate/up accumulate over ht in PSUM; silu (ACT) * up (DVE) ->
        hact[ft] bf16 in SBUF.
      for ct (128-token blocks): for hh (two 512 halves):
        down accumulated over 16 ft in PSUM -> copy -> DMA y [C, H] fp32.
  - Host: out[tokens_e] += down_e * cw_e.
"""

import numpy as np
from contextlib import ExitStack

N_CORES = 8
B, S, H = 4, 2048, 1024
F, E = 2048, 16
T = B * S
N_SLOTS = 2  # experts per core

P = 128
HT = H // P   # 8  h-tiles
FT = F // P   # 16 f-tiles

TRACE = False          # test.py sets this for profiled runs
TRACE_CORES = [7]      # core-0 NTFF capture crashes fast kernels here
LAST_RESULT = None     # BassKernelResults of last run (for test.py)

_nc_cache = {}


def _parts(s):
    """Split s columns into matmul parts, each <=512 and >=256 where
    possible (moving dim >=256 keeps full PE rate)."""
    out = []
    while s > 768:
        out.append(512)
        s -= 512
    if s > 512:
        out.append(s - 256)
        out.append(256)
    elif s > 0:
        out.append(s)
    return out


def _pairs(C):
    """Column ranges grouped <=2 parts each, so G12 holds at most
    4 PSUM banks (2 parts x gate+up) at a time."""
    parts = _parts(C)
    groups = []
    o = 0
    for i in range(0, len(parts), 2):
        ps = parts[i:i + 2]
        offs = []
        for p_ in ps:
            offs.append((o, p_))
            o += p_
        groups.append(offs)
    return groups


def _build_nc(caps):
    import concourse.tile as tile
    from concourse import bacc, mybir

    nc = bacc.Bacc("TRN2", target_bir_lowering=False, debug=False,
                   enable_asserts=False, num_devices=N_CORES)
    dt = mybir.dt.float32
    mdt = mybir.dt.bfloat16
    SILU = mybir.ActivationFunctionType.Silu

    xts, w1ts, v1ts, w2s, ys = [], [], [], [], []
    for s in range(N_SLOTS):
        C = caps[s]
        xts.append(nc.dram_tensor(f"xt{s}", [H, C], mdt,
                                  kind="ExternalInput").ap())
        # w1t/v1t pre-blocked: [ft, p(h%128), o(h//128), f] so each ft
        # slice is contiguous and DMAs as 128 x 2KB descriptors
        w1ts.append(nc.dram_tensor(f"w1t{s}", [FT, P, HT, P], mdt,
                                   kind="ExternalInput").ap())
        v1ts.append(nc.dram_tensor(f"v1t{s}", [FT, P, HT, P], mdt,
                                   kind="ExternalInput").ap())
        w2s.append(nc.dram_tensor(f"w2_{s}", [F, H], mdt,
                                  kind="ExternalInput").ap())
        ys.append(nc.dram_tensor(f"y{s}", [C, H], dt,
                                 kind="ExternalOutput").ap())

    with tile.TileContext(nc) as tc:
        with ExitStack() as ctx:
            # 16 xt bufs: both slots' tiles resident so slot1's DMA
            # streams during slot0 compute
            xt_pool = ctx.enter_context(tc.tile_pool(name="xt", bufs=2 * HT))
            wst_pool = ctx.enter_context(tc.tile_pool(name="wst", bufs=4))
            w2_pool = ctx.enter_context(tc.tile_pool(name="w2sb", bufs=FT))
            hact_pool = ctx.enter_context(tc.tile_pool(name="hact", bufs=FT))
            silu_pool = ctx.enter_context(tc.tile_pool(name="silu", bufs=3))
            out_pool = ctx.enter_context(tc.tile_pool(name="out", bufs=2))
            ps_pool = ctx.enter_context(tc.tile_pool(name="ps", bufs=8, space="PSUM"))

            for s in range(N_SLOTS):
                C = caps[s]
                xt, w1t, v1t, w2, y = xts[s], w1ts[s], v1ts[s], w2s[s], ys[s]
                groups = _pairs(C)

                # ft0 weights land before the xt block: the first matmul
                # waits on ~0.75MB of DMA, then consumes xt ht-by-ht
                w1s0 = wst_pool.tile([P, HT, P], mdt, tag="wst")
                v1s0 = wst_pool.tile([P, HT, P], mdt, tag="wst")
                nc.sync.dma_start(w1s0[:], w1t[0])
                nc.sync.dma_start(v1s0[:], v1t[0])

                xt_sb = []
                for ht in range(HT):
                    t = xt_pool.tile([P, C], mdt, tag="xt")
                    nc.sync.dma_start(t[:], xt[ht * P:(ht + 1) * P, :])
                    xt_sb.append(t)

                # GEMM1/2 + GLU, ft-outer: w1/v1 stream exactly once
                hact_sb = []
                for ft in range(FT):
                    h_t = hact_pool.tile([P, C], mdt, tag="hact")
                    if ft == 0:
                        w1s, v1s = w1s0, v1s0
                    else:
                        w1s = wst_pool.tile([P, HT, P], mdt, tag="wst")
                        v1s = wst_pool.tile([P, HT, P], mdt, tag="wst")
                        nc.sync.dma_start(w1s[:], w1t[ft])
                        nc.sync.dma_start(v1s[:], v1t[ft])
                    for offs in groups:
                        g_tiles = [ps_pool.tile([P, p_], dt, tag="ps",
                                                name=f"g{i_}")
                                   for i_, (_, p_) in enumerate(offs)]
                        u_tiles = [ps_pool.tile([P, p_], dt, tag="ps",
                                               name=f"u{i_}")
                                   for i_, (_, p_) in enumerate(offs)]
                        for ht in range(HT):
                            for i_, (o_, p_) in enumerate(offs):
                                nc.tensor.matmul(
                                    g_tiles[i_][:], w1s[:, ht, :],
                                    xt_sb[ht][:, o_:o_ + p_],
                                    start=(ht == 0), stop=(ht == HT - 1))
                            for i_, (o_, p_) in enumerate(offs):
                                nc.tensor.matmul(
                                    u_tiles[i_][:], v1s[:, ht, :],
                                    xt_sb[ht][:, o_:o_ + p_],
                                    start=(ht == 0), stop=(ht == HT - 1))
                        for i_, (o_, p_) in enumerate(offs):
                            sl = silu_pool.tile([P, p_], dt, tag="sl")
                            nc.scalar.activation(sl[:], g_tiles[i_][:], SILU)
                            nc.vector.tensor_mul(
                                h_t[:, o_:o_ + p_], sl[:], u_tiles[i_][:])
                    hact_sb.append(h_t)
                    if ft == 0:
                        # w2 queued behind ft1's weights; arrives well
                        # before GEMM3 needs it
                        w2_sb = []
                        for f2 in range(FT):
                            t = w2_pool.tile([P, H], mdt, tag="w2",
                                             name=f"w2_{f2}")
                            nc.sync.dma_start(
                                t[:], w2[f2 * P:(f2 + 1) * P, :])
                            w2_sb.append(t)

                # GEMM3: down[c, h] accumulated over f-tiles; H split into
                # two 512 halves so psum slots stay one bank each
                for ct in range(-(-C // P)):
                    cp = min(P, C - ct * P)
                    o_t = out_pool.tile([P, H], dt, tag="o")
                    for hi, hh in enumerate(range(0, H, 512)):
                        d_ps = ps_pool.tile([P, 512], dt, tag="ps",
                                            name=f"d{hi}")
                        for ft in range(FT):
                            nc.tensor.matmul(
                                d_ps[:cp, :],
                                hact_sb[ft][:, ct * P:ct * P + cp],
                                w2_sb[ft][:, hh:hh + 512],
                                start=(ft == 0), stop=(ft == FT - 1))
                        nc.any.tensor_copy(o_t[:cp, hh:hh + 512],
                                           d_ps[:cp, :])
                        nc.sync.dma_start(
                            y[ct * P:ct * P + cp, hh:hh + 512],
                            o_t[:cp, hh:hh + 512])
    nc.compile()
    return nc


def _get_nc(caps):
    if caps not in _nc_cache:
        _nc_cache[caps] = _build_nc(caps)
    return _nc_cache[caps]


def prepare(x, top_weights, top_experts, w1, v1, w2):
    """Host-side routing + sharded input construction."""
    import ml_dtypes
    bf16 = ml_dtypes.bfloat16
    x = np.asarray(x, dtype=np.float32)
    top_weights = np.asarray(top_weights, dtype=np.float32)
    top_experts = np.asarray(top_experts).astype(np.int64)

    xf = x.reshape(T, H)

    # combine weights per (token, expert); duplicate slots sum
    cw = np.zeros((T, E), dtype=np.float32)
    np.add.at(cw, (np.arange(T)[:, None], top_experts), top_weights)

    idx = [np.nonzero(cw[:, e])[0] for e in range(E)]
    counts = [len(i) for i in idx]

    order = sorted(range(E), key=lambda e: -counts[e])
    slot_experts = [[order[m] for m in range(N_CORES)],
                    [order[2 * N_CORES - 1 - m] for m in range(N_CORES)]]
    # even capacities keep all matmul free sizes even (ISA-safe)
    caps = tuple(max(256, (max(counts[e] for e in slot_experts[s]) + 1)
                 // 2 * 2) for s in range(N_SLOTS))

    def _block(we):
        # [F, H] -> [ft, p(h%128), o(h//128), f], bf16
        wl = np.asarray(we, dtype=np.float32).reshape(FT, P, HT, P)
        return np.ascontiguousarray(
            wl.transpose(0, 3, 2, 1)).astype(bf16)

    in_maps = []
    for m in range(N_CORES):
        im = {}
        for s in range(N_SLOTS):
            e = slot_experts[s][m]
            C = caps[s]
            XT = np.zeros((H, C), dtype=bf16)
            XT[:, :counts[e]] = xf[idx[e]].T.astype(bf16)
            im[f"xt{s}"] = XT
            im[f"w1t{s}"] = _block(w1[e])
            im[f"v1t{s}"] = _block(v1[e])
            im[f"w2_{s}"] = np.ascontiguousarray(
                np.asarray(w2[e], dtype=np.float32)).astype(bf16)
        in_maps.append(im)
    return caps, in_maps, slot_experts, idx, counts, cw


def combine(results, slot_experts, idx, counts, cw):
    """Weighted scatter-add of per-core expert outputs into [B, S, H]."""
    out = np.zeros((T, H), dtype=np.float32)
    for m in range(N_CORES):
        for s in range(N_SLOTS):
            e = slot_experts[s][m]
            n = counts[e]
            if n:
                ym = results[m][f"y{s}"]
                out[idx[e]] += ym[:n, :] * cw[idx[e], e][:, None]
    return out.reshape(B, S, H)


def kernel(x, weights, top_weights, top_experts, w1, v1, w2):
    global LAST_RESULT
    caps, in_maps, slot_experts, idx, counts, cw = prepare(
        x, top_weights, top_experts, w1, v1, w2)
    nc = _get_nc(caps)
    from concourse.bass_utils import run_bass_kernel_spmd
    res = run_bass_kernel_spmd(nc, in_maps, list(range(N_CORES)), trace=TRACE,
                               trace_cores=TRACE_CORES if TRACE else None)
    LAST_RESULT = res
    return combine(res.results, slot_experts, idx, counts, cw)
